# revision 37
# baseline (speedup 1.0000x reference)
"""Trainium2 Bass kernel for nn_Correlation (FlowNet-style 1-D correlation).

out[b, d, h, w] = mean_c( left[b,c,h,w] * right[b,c,h,w+d-40] ), d in [0,81),
with right zero-padded along W.  Inputs left/right: [4, 256, 128, 416] fp32.

Strategy (the 512 (b,h) rows are sharded over 8 cores by H, 16 rows each):
  * out[:, :, h, :] is the 81-wide band of the Gram matrix
    G[w, w'] = sum_c L[c, w] R[c, w'] (contraction C=256 = 2x128 partition
    halves accumulated in fp32 PSUM).  Each W-tile of L (widths 128/128/128/32)
    is the PE stationary; a clipped window of R columns streams through
    (window widths 168/208/200/72 = 648 columns per C-half per h-row).
  * Inputs are cast to fp16 on the host: halves HBM traffic and runs the PE
    at 1 cycle/column.
  * PSUM packing: j0+j1 share one bank (cols 0:168 / 168:376); j2 gets a
    second bank with j3's [32, 72] output folded into j2's provably-dead
    corner [0:32, 128:200] (there c - p > 80, outside the band).  One ACT
    eviction (376 cols) + one DVE eviction (200 cols) per h-row, cast to
    fp16 with the 1/C mean scale fused, into a [128, nh, 576] store tile.
  * slab-64 eviction compaction (SLABS64): each window is evicted in two
    64-row groups with group k's columns shifted left by 64k, narrowing each
    out section to 63+81=144 cols ([168,208,200]+fold -> [144,144,144,72],
    576 -> 504).  Band-diagonal extraction -- which on-device would need
    either a sheared DRAM bounce (~20 MB extra HBM traffic per core) or
    per-partition diagonal APs that walrus rejects on every engine --
    happens on the HOST with one vectorized numpy gather (deshear()).
  * Per-core HBM traffic is 27.2 MB in + 8.3 MB out at ~348 GB/s/core
    (nh=16 -> 3.4 MB input DMAs), the binding roofline; PE (~40 us),
    ACT (~51 us) and DVE (~62 us) sit under the ~100 us DMA wall.
    Measured true per-rep ~101 us vs ~291 us for the sheared-bounce
    baseline (both reps-delta, r128-r32).
"""

import sys

sys.path.insert(0, "/opt/trn_rl_repo")

from contextlib import ExitStack

import numpy as np

import concourse.bass as bass
import concourse.tile as tile
from concourse import mybir

B, C, H, W = 4, 256, 128, 416
MD = 40
D = 2 * MD + 1  # 81 displacement channels
NCORES = 8
HS = H // NCORES  # 16 H-rows per core

W0S = [0, 128, 256, 384]  # w-tile starts
MS = [128, 128, 128, 32]  # w-tile widths

NH = 16  # h-rows per chunk (input DMA / output DMA batch)
BUFS = {"inp": 2, "work": 2, "psg": 4}  # psg: 2 tags (gA,gB) x 4 bufs = 8 banks


def _windows():
    """Per-tile R-stream windows over unpadded right coords.

    Returns (a_j, n_j): stream start/len in right cols.  Band entry (p, d)
    of tile j sits at window col c = p + (r0_j - a_j) + d when in range.
    """
    res = []
    for w0, m in zip(W0S, MS):
        r0 = w0 - MD
        lo = max(0, -r0)
        hi = min(m + 2 * MD, W - r0)
        res.append((r0 + lo, hi - lo))
    return res


WINS = _windows()  # [(0,168), (88,208), (216,200), (344,72)]
NJ = [n for _, n in WINS]
# j3's [32, 72] band is packed into j2's PSUM bank at [0:32, 128:200] -- that
# rectangle is provably outside j2's band (c - p > 80 for p < 32, c >= 128),
# so the j2 eviction/store carries j3 for free.  Sections: [168, 208, 200].
CUM = [0, NJ[0], NJ[0] + NJ[1], NJ[0] + NJ[1] + NJ[2]]
WIN = CUM[-1]  # 576 window columns per h-row

# slab-64 compaction: evict each tile's PSUM window in two 64-row groups,
# shifting group k's columns left by 64k, so each section narrows to
# 63+81=144 cols (the band spans [p%64, p%64+80] there).  j3 ([32, 72]) gets
# its own section and its own gA columns [376:448].  Out shrinks 576->504.
CUM64 = [0, 144, 288, 432]
WIN64 = 504
# eviction slabs: (bank, p0, p1, src_c0, width, dst_col, engine)
#   bank: 0=gA, 1=gB; dst_col is absolute in the [128, WIN64] store tile
SLABS64 = [
    (0, 0, 64, 0, 104, CUM64[0] + 40, "v"),       # j0 rows 0:64,  c [0,104)
    (0, 64, 128, 24, 144, CUM64[0] + 0, "v"),     # j0 rows 64:,   c [24,168)
    (0, 0, 64, NJ[0] + 0, 144, CUM64[1] + 0, "s"),    # j1 rows 0:64
    (0, 64, 128, NJ[0] + 64, 144, CUM64[1] + 0, "s"),  # j1 rows 64:
    (1, 0, 64, 0, 144, CUM64[2] + 0, "s"),        # j2 rows 0:64
    (1, 64, 128, 64, 136, CUM64[2] + 0, "v"),     # j2 rows 64:
    (0, 0, 32, 376, 72, CUM64[3] + 0, "v"),       # j3
]

# slab-32 compaction: four 32-row groups per tile, sections 31+81=112 wide
# ([112, 112, 112, 72], WIN32=408).  Slabs sharing (bank, rows, width) fuse
# into ONE eviction via a 3-level AP [[partition], [block], [elem]]:
#   F1 = j0k0 + j3 (rows 0:32, w72), F2 = j0k2 + j1k2, F3 = j0k3 + j1k3.
# j3's section (only rows 0:32 live) is stored by its own small DMA, so the
# main store is [128, nh, 336] and out drops 504 -> 336+72/4 rows-equiv.
CUM32 = [0, 112, 224, 336]
WIN32 = 408
# (bank, p0, p1, [src_c0...], [dst_c0...], width, engine)
SLABS32 = [
    (0, 0, 32, [0, 376], [40, 336], 72, "v"),         # F1: j0k0 + j3
    (0, 32, 64, [0], [8], 104, "v"),                  # j0k1 (src clip at 0)
    (0, 64, 96, [24, NJ[0] + 64], [0, 112], 112, "s"),    # F2: j0k2 + j1k2
    (0, 96, 128, [56, NJ[0] + 96], [0, 112], 112, "s"),   # F3: j0k3 + j1k3
    (0, 0, 32, [NJ[0]], [112], 112, "s"),             # j1k0
    (0, 32, 64, [NJ[0] + 32], [112], 112, "v"),       # j1k1
    (1, 0, 32, [0], [224], 112, "s"),                 # j2k0
    (1, 32, 64, [32], [224], 112, "v"),               # j2k1
    (1, 64, 96, [64], [224], 112, "v"),               # j2k2
    (1, 96, 128, [96], [224], 104, "s"),              # j2k3 (clip at 200)
]


def _blocks_ap(t_slice, c0s, width):
    """3-level AP [[partition], [block], [elem]] selecting equal-width column
    blocks at c0s from a 2D [partitions, cols] slice."""
    src = t_slice[:, c0s[0] : c0s[0] + width]
    dims = src.ap
    d0 = list(dims[0])
    dims.clear()
    if len(c0s) == 1:
        dims.extend([d0, [1, width]])
    else:
        dims.extend([d0, [c0s[1] - c0s[0], len(c0s)], [1, width]])
    src.ap = dims
    return src


COMPACTION = "g64"  # measured: g64 ~99us/rep beats g32 ~138us (false-dep
# serialization of the 10-instr fused evictions) and plain ~108us


def corr_kernel(
    tc, outs, ins, hs=HS, nh=NH, bufs=None, reps=1,
    do_load=True, do_compute=True, do_store=True, compaction=COMPACTION,
):
    nc = tc.nc
    left, right = ins["left"], ins["right"]
    wins_out = outs["wins"]
    wins3_out = outs.get("wins3")
    bufs = bufs or {}
    hc_n = hs // nh
    if compaction is True:
        compaction = "g64"
    win = {"g32": WIN32, "g64": WIN64}.get(compaction, WIN)
    assert hs % nh == 0

    with ExitStack() as ctx:
        inp = ctx.enter_context(tc.tile_pool(name="inp", bufs=bufs.get("inp", 3)))
        work = ctx.enter_context(tc.tile_pool(name="work", bufs=bufs.get("work", 3)))
        psg = ctx.enter_context(
            tc.tile_pool(name="psg", bufs=bufs.get("psg", 8), space="PSUM")
        )

        def one_chunk(b, hc):
            L4 = inp.tile([128, 2, nh * W], mybir.dt.float16, tag="L")
            R4 = inp.tile([128, 2, nh * W], mybir.dt.float16, tag="R")
            if not do_load:  # probe mode: satisfy write-before-read cheaply
                nc.vector.memset(L4[:, :, 0:4], 0.0)
                nc.vector.memset(R4[:, :, 0:4], 0.0)
            if do_load:
                nc.sync.dma_start(
                    L4[:],
                    left[b, :, hc * nh : (hc + 1) * nh, :].rearrange(
                        "(t p) h w -> p t (h w)", p=128
                    ),
                )
                nc.sync.dma_start(
                    R4[:],
                    right[b, :, hc * nh : (hc + 1) * nh, :].rearrange(
                        "(t p) h w -> p t (h w)", p=128
                    ),
                )

            Bt = work.tile([128, nh, win], mybir.dt.float16, tag="B")
            if not do_compute:  # probe mode: satisfy write-before-read
                nc.vector.memset(Bt[:, 0, 0:4], 0.0)
            if do_compute:
                for hl in range(nh):
                    hw0 = hl * W
                    # bank A: j0 at cols [0:168], j1 at [168:376],
                    #         (compaction: j3 at [0:32, 376:448])
                    gA = psg.tile([128, 512], mybir.dt.float32, tag="gA")
                    # bank B: j2 at [0:200]
                    #         (no compaction: j3 folded at [0:32, 128:200])
                    gB = psg.tile([128, 512], mybir.dt.float32, tag="gB")
                    c3 = 376 if compaction else 128
                    for j in range(4):
                        a, n = WINS[j]
                        dst_g = (gA, gA, gB, gA if compaction else gB)[j]
                        c0 = (0, NJ[0], 0, c3)[j]
                        for t in range(2):
                            lhsT = L4[:, t, hw0 + W0S[j] : hw0 + W0S[j] + MS[j]]
                            rhs = R4[:, t, hw0 + a : hw0 + a + n]
                            nc.tensor.matmul(
                                dst_g[0 : MS[j], c0 : c0 + n], lhsT, rhs,
                                start=(t == 0), stop=(t == 1),
                            )
                    if compaction == "g32":
                        for bk, p0, p1, scs, dcs, wd, eng in SLABS32:
                            src = _blocks_ap((gA, gB)[bk][p0:p1, :], scs, wd)
                            dst = _blocks_ap(Bt[p0:p1, hl, :], dcs, wd)
                            if eng == "s":
                                nc.scalar.mul(dst, src, 1.0 / C)
                            else:
                                nc.vector.tensor_scalar_mul(dst, src, 1.0 / C)
                    elif compaction == "g64":
                        for bk, p0, p1, sc, wd, dc, eng in SLABS64:
                            src = (gA, gB)[bk][p0:p1, sc : sc + wd]
                            dst = Bt[p0:p1, hl, dc : dc + wd]
                            if eng == "s":
                                nc.scalar.mul(dst, src, 1.0 / C)
                            else:
                                nc.vector.tensor_scalar_mul(dst, src, 1.0 / C)
                    else:
                        # bank A (376 cols) on ACT, bank B (200 cols) on DVE
                        nc.scalar.mul(
                            Bt[:, hl, 0 : CUM[2]], gA[:, 0 : CUM[2]], 1.0 / C
                        )
                        nc.vector.tensor_scalar_mul(
                            Bt[:, hl, CUM[2] : WIN], gB[:, 0 : WIN - CUM[2]],
                            1.0 / C,
                        )

            if do_store:
                # issue on the ACT HWDGE ring: its wait (Bt complete) resolves
                # right as ACT's own last eviction retires, so it stalls
                # nothing, and the sync ring stays free for input prefetch
                if compaction == "g32":
                    # j3's section holds live data only in rows 0:32 -- store
                    # it separately and skip its 96 dead rows (-1.1 MB/core)
                    nc.scalar.dma_start(wins_out[b, hc], Bt[:, :, 0 : CUM32[3]])
                    nc.scalar.dma_start(
                        wins3_out[b, hc], Bt[0:32, :, CUM32[3] : WIN32]
                    )
                else:
                    nc.scalar.dma_start(wins_out[b, hc], Bt[:])

        for _rep in range(reps):
            for b in range(B):
                for hc in range(hc_n):
                    one_chunk(b, hc)


def split_dma_waits(nc):
    """Legalize for walrus: instruction descriptors hold ONE sync wait
    (NEURON_ISA_TPB_EVENTS), but Tile attaches up to ~3.  Move the extras to
    standalone waits on the instruction's engine right before it --
    sequencers execute (and enqueue HWDGE descriptors) in program order, so
    the hoisted waits still guard the instruction."""
    n = 0
    for fn in nc.m.functions:
        for bb in fn.blocks:
            insts = bb.instructions
            out = []
            for inst in insts:
                si = getattr(inst, "sync_info", None)
                eng = getattr(inst, "engine", None)
                if (
                    si is not None
                    and si.on_wait
                    and len(si.on_wait) > 1
                    and eng is not None
                    and eng != mybir.EngineType.Unassigned
                ):
                    waits = list(si.on_wait)
                    for w in waits[:-1]:
                        ev = mybir.InstNoOp(name=f"{inst.name}-prewait{n}")
                        ev.engine = eng
                        ev.sync_info = mybir.SyncInfo(on_wait=[w], on_update=[])
                        nc.register_instruction(ev)
                        out.append(ev)
                        n += 1
                    inst.sync_info = mybir.SyncInfo(
                        on_wait=waits[-1:], on_update=list(si.on_update or [])
                    )
                out.append(inst)
            bb.instructions = out
    return n


def build_nc(hs=HS, nh=NH, reps=1, bufs=None, compaction=COMPACTION, **kw):
    if compaction is True:
        compaction = "g64"
    nc = bass.Bass(
        trn_type="TRN2", target_bir_lowering=False, debug=False, num_devices=NCORES
    )
    ins = {
        "left": nc.dram_tensor(
            "left", [B, C, hs, W], mybir.dt.float16, kind="ExternalInput"
        ).ap(),
        "right": nc.dram_tensor(
            "right", [B, C, hs, W], mybir.dt.float16, kind="ExternalInput"
        ).ap(),
    }
    main_win = {"g32": CUM32[3], "g64": WIN64}.get(compaction, WIN)
    outs = {
        "wins": nc.dram_tensor(
            "wins",
            [B, hs // nh, 128, nh, main_win],
            mybir.dt.float16,
            kind="ExternalOutput",
        ).ap()
    }
    if compaction == "g32":
        outs["wins3"] = nc.dram_tensor(
            "wins3",
            [B, hs // nh, 32, nh, WIN32 - CUM32[3]],
            mybir.dt.float16,
            kind="ExternalOutput",
        ).ap()
    with tile.TileContext(nc) as tc:
        corr_kernel(
            tc, outs, ins, hs=hs, nh=nh, bufs=bufs or BUFS, reps=reps,
            compaction=compaction, **kw
        )
    split_dma_waits(nc)
    return nc


def make_in_maps(left, right):
    in_maps = []
    for i in range(NCORES):
        sl = slice(i * HS, (i + 1) * HS)
        in_maps.append(
            {
                "left": np.ascontiguousarray(left[:, :, sl, :]).astype(np.float16),
                "right": np.ascontiguousarray(right[:, :, sl, :]).astype(np.float16),
            }
        )
    return in_maps


def _deshear_luts(compaction=COMPACTION):
    """Host-side gather LUTs: out[b,d,h,w] = V[b,h, P[w], COL[d,w]] * MASK."""
    w = np.arange(W)
    d = np.arange(D)
    p = w % 128
    if compaction == "g32":
        j = w // 128  # j3 has its own section; p%32 == p there
        col = np.asarray(CUM32)[j][None, :] + (p % 32)[None, :] + d[:, None]
        win = WIN32
    elif compaction == "g64" or compaction is True:
        j = w // 128  # j3 has its own section
        col = np.asarray(CUM64)[j][None, :] + (p % 64)[None, :] + d[:, None]
        win = WIN64
    else:
        j = np.minimum(w // 128, 2)  # j3 lives inside section 2 (cols 128+)
        delta = np.where(j == 0, -MD, 0)
        cum = np.asarray(CUM[:3])
        col = (
            cum[j][None, :] + (w - 128 * j)[None, :] + delta[None, :]
            + d[:, None]
        )
        win = WIN
    r = w[None, :] + d[:, None] - MD
    mask = (r >= 0) & (r < W)
    col = np.clip(col, 0, win - 1)
    return p, col, mask


def deshear(wins_all, hs=HS, nh=NH, compaction=COMPACTION, wins3_all=None):
    """wins_all: [ncores, B, hs//nh, 128, nh, win] fp16 -> [B, D, H, W] f32.

    For g32, wins_all is the [.., 336]-wide main store and wins3_all the
    [.., 32, nh, 72] j3 store; they are re-joined into a [.., 128, 408] view.
    """
    ncores, nb = wins_all.shape[0], wins_all.shape[1]
    if compaction == "g32":
        assert wins3_all is not None
        hc_n = wins_all.shape[2]
        full = np.empty(
            (ncores, nb, hc_n, 128, nh, WIN32), wins_all.dtype
        )
        full[..., 0 : CUM32[3]] = wins_all
        full[:, :, :, 0:32, :, CUM32[3] :] = wins3_all
        wins_all = full
    win = wins_all.shape[-1]
    p, col, mask = _deshear_luts(compaction)
    v = wins_all.transpose(0, 1, 2, 4, 3, 5).reshape(ncores, nb, hs, 128, win)
    pb = np.broadcast_to(p[None, :], (D, W))
    res = v[:, :, :, pb, col]  # [ncores, nb, hs, D, W]
    # np.where (not multiply): masked-off slots may hold uninitialized SBUF
    # garbage, which can be NaN
    res = np.where(mask[None, None, None], res.astype(np.float32), 0.0)
    return res.transpose(1, 3, 0, 2, 4).reshape(nb, D, ncores * hs, W)


def kernel(left, right):
    """Full-input entry point: [4,256,128,416] fp32 x2 -> [4,81,128,416] fp32."""
    from concourse.bass_utils import run_bass_kernel_spmd

    left = np.asarray(left, dtype=np.float32)
    right = np.asarray(right, dtype=np.float32)
    nc = build_nc()
    in_maps = make_in_maps(left, right)
    res = run_bass_kernel_spmd(nc, in_maps, list(range(NCORES)))
    wins_all = np.stack([res.results[i]["wins"] for i in range(NCORES)])
    wins3_all = None
    if COMPACTION == "g32":
        wins3_all = np.stack([res.results[i]["wins3"] for i in range(NCORES)])
    return deshear(wins_all, wins3_all=wins3_all)


if __name__ == "__main__":
    rng = np.random.default_rng(0)
    lf = rng.standard_normal((B, C, H, W), dtype=np.float32)
    rt = rng.standard_normal((B, C, H, W), dtype=np.float32)
    o = kernel(left=lf, right=rt)
    print(o.shape, o.dtype)


# revision 50
# speedup vs baseline: 5.2948x; 5.2948x over previous
"""Trainium2 Bass kernel for nn_Correlation (FlowNet-style 1-D correlation).

out[b, d, h, w] = mean_c( left[b,c,h,w] * right[b,c,h,w+d-40] ), d in [0,81),
with right zero-padded along W.  Inputs left/right: [4, 256, 128, 416] fp32.

Strategy (the 512 (b,h) rows are sharded over 8 cores by H, 16 rows each):
  * out[:, :, h, :] is the 81-wide band of the Gram matrix
    G[w, w'] = sum_c L[c, w] R[c, w'] (contraction C=256 = 2x128 partition
    halves accumulated in fp32 PSUM).  Each W-tile of L (widths 128/128/128/32)
    is the PE stationary; a clipped window of R columns streams through
    (window widths 168/208/200/72 = 648 columns per C-half per h-row).
  * Inputs are cast to fp16 on the host: halves HBM traffic and runs the PE
    at 1 cycle/column.
  * PSUM packing: j0+j1 share one bank (cols 0:168 / 168:376); j2 gets a
    second bank with j3's [32, 72] output folded into j2's provably-dead
    corner [0:32, 128:200] (there c - p > 80, outside the band).  One ACT
    eviction (376 cols) + one DVE eviction (200 cols) per h-row, cast to
    fp16 with the 1/C mean scale fused, into a [128, nh, 576] store tile.
  * slab-64 eviction compaction (SLABS64): each window is evicted in two
    64-row groups with group k's columns shifted left by 64k, narrowing each
    out section to 63+81=144 cols ([168,208,200]+fold -> [144,144,144,72],
    576 -> 504).  Finer variants measured SLOWER: slab-32 fused evictions
    (138us vs 99us; Tile's span-conservative deps serialize the denser
    schedule) and a split store skipping j3's dead rows (136us; the tiny
    second DMA per chunk beats its 1.09MB saving).  Fewest/widest copies
    and stores win.  Band-diagonal extraction -- which on-device would need
    either a sheared DRAM bounce (~20 MB extra HBM traffic per core) or
    per-partition diagonal APs that walrus rejects on every engine --
    happens on the HOST with one vectorized numpy gather (deshear()).
  * Per-core HBM traffic is 27.2 MB in + 8.3 MB out at ~348 GB/s/core
    (nh=16 -> 3.4 MB input DMAs), the binding roofline; PE (~40 us),
    ACT (~51 us) and DVE (~62 us) sit under the ~100 us DMA wall.
    Measured true per-rep ~101 us vs ~291 us for the sheared-bounce
    baseline (both reps-delta, r128-r32).
"""

import sys

sys.path.insert(0, "/opt/trn_rl_repo")

from contextlib import ExitStack

import numpy as np

import concourse.bass as bass
import concourse.tile as tile
from concourse import mybir

B, C, H, W = 4, 256, 128, 416
MD = 40
D = 2 * MD + 1  # 81 displacement channels
NCORES = 8
HS = H // NCORES  # 16 H-rows per core

W0S = [0, 128, 256, 384]  # w-tile starts
MS = [128, 128, 128, 32]  # w-tile widths

NH = 16  # h-rows per chunk (input DMA / output DMA batch)
BUFS = {"inp": 2, "work": 2, "psg": 4}  # psg: 2 tags (gA,gB) x 4 bufs = 8 banks


def _windows():
    """Per-tile R-stream windows over unpadded right coords.

    Returns (a_j, n_j): stream start/len in right cols.  Band entry (p, d)
    of tile j sits at window col c = p + (r0_j - a_j) + d when in range.
    """
    res = []
    for w0, m in zip(W0S, MS):
        r0 = w0 - MD
        lo = max(0, -r0)
        hi = min(m + 2 * MD, W - r0)
        res.append((r0 + lo, hi - lo))
    return res


WINS = _windows()  # [(0,168), (88,208), (216,200), (344,72)]
NJ = [n for _, n in WINS]
# j3's [32, 72] band is packed into j2's PSUM bank at [0:32, 128:200] -- that
# rectangle is provably outside j2's band (c - p > 80 for p < 32, c >= 128),
# so the j2 eviction/store carries j3 for free.  Sections: [168, 208, 200].
CUM = [0, NJ[0], NJ[0] + NJ[1], NJ[0] + NJ[1] + NJ[2]]
WIN = CUM[-1]  # 576 window columns per h-row

# slab-64 compaction: evict each tile's PSUM window in two 64-row groups,
# shifting group k's columns left by 64k, so each section narrows to
# 63+81=144 cols (the band spans [p%64, p%64+80] there).  j3 ([32, 72]) gets
# its own section and its own gA columns [376:448].  Out shrinks 576->504.
CUM64 = [0, 144, 288, 432]
WIN64 = 504
# eviction slabs: (bank, p0, p1, src_c0, width, dst_col, engine)
#   bank: 0=gA, 1=gB; dst_col is absolute in the [128, WIN64] store tile
SLABS64 = [
    (0, 0, 64, 0, 104, CUM64[0] + 40, "v"),       # j0 rows 0:64,  c [0,104)
    (0, 64, 128, 24, 144, CUM64[0] + 0, "v"),     # j0 rows 64:,   c [24,168)
    (0, 0, 64, NJ[0] + 0, 144, CUM64[1] + 0, "s"),    # j1 rows 0:64
    (0, 64, 128, NJ[0] + 64, 144, CUM64[1] + 0, "s"),  # j1 rows 64:
    (1, 0, 64, 0, 144, CUM64[2] + 0, "s"),        # j2 rows 0:64
    (1, 64, 128, 64, 136, CUM64[2] + 0, "v"),     # j2 rows 64:
    (0, 0, 32, 376, 72, CUM64[3] + 0, "v"),       # j3
]

# slab-32 compaction: four 32-row groups per tile, sections 31+81=112 wide
# ([112, 112, 112, 72], WIN32=408).  Slabs sharing (bank, rows, width) fuse
# into ONE eviction via a 3-level AP [[partition], [block], [elem]]:
#   F1 = j0k0 + j3 (rows 0:32, w72), F2 = j0k2 + j1k2, F3 = j0k3 + j1k3.
# j3's section (only rows 0:32 live) is stored by its own small DMA, so the
# main store is [128, nh, 336] and out drops 504 -> 336+72/4 rows-equiv.
CUM32 = [0, 112, 224, 336]
WIN32 = 408
# (bank, p0, p1, [src_c0...], [dst_c0...], width, engine)
SLABS32 = [
    (0, 0, 32, [0, 376], [40, 336], 72, "v"),         # F1: j0k0 + j3
    (0, 32, 64, [0], [8], 104, "v"),                  # j0k1 (src clip at 0)
    (0, 64, 96, [24, NJ[0] + 64], [0, 112], 112, "s"),    # F2: j0k2 + j1k2
    (0, 96, 128, [56, NJ[0] + 96], [0, 112], 112, "s"),   # F3: j0k3 + j1k3
    (0, 0, 32, [NJ[0]], [112], 112, "s"),             # j1k0
    (0, 32, 64, [NJ[0] + 32], [112], 112, "v"),       # j1k1
    (1, 0, 32, [0], [224], 112, "s"),                 # j2k0
    (1, 32, 64, [32], [224], 112, "v"),               # j2k1
    (1, 64, 96, [64], [224], 112, "v"),               # j2k2
    (1, 96, 128, [96], [224], 104, "s"),              # j2k3 (clip at 200)
]


def _blocks_ap(t_slice, c0s, width):
    """3-level AP [[partition], [block], [elem]] selecting equal-width column
    blocks at c0s from a 2D [partitions, cols] slice."""
    src = t_slice[:, c0s[0] : c0s[0] + width]
    dims = src.ap
    d0 = list(dims[0])
    dims.clear()
    if len(c0s) == 1:
        dims.extend([d0, [1, width]])
    else:
        dims.extend([d0, [c0s[1] - c0s[0], len(c0s)], [1, width]])
    src.ap = dims
    return src


COMPACTION = "g64"  # measured champion at ~92-101us/rep.  Rejected:
# plain (108us), g32 fused-slab evictions (138us: span-conservative deps
# serialize 10 copies/row), g64s j3-split store (136us: the tiny 144B-run
# second DMA per chunk costs far more than the 1.09MB it saves).


def corr_kernel(
    tc, outs, ins, hs=HS, nh=NH, bufs=None, reps=1,
    do_load=True, do_compute=True, do_store=True, compaction=COMPACTION,
):
    nc = tc.nc
    left, right = ins["left"], ins["right"]
    wins_out = outs["wins"]
    wins3_out = outs.get("wins3")
    bufs = bufs or {}
    hc_n = hs // nh
    if compaction is True:
        compaction = "g64"
    win = {"g32": WIN32, "g64": WIN64, "g64s": WIN64}.get(compaction, WIN)
    assert hs % nh == 0

    with ExitStack() as ctx:
        inp = ctx.enter_context(tc.tile_pool(name="inp", bufs=bufs.get("inp", 3)))
        work = ctx.enter_context(tc.tile_pool(name="work", bufs=bufs.get("work", 3)))
        psg = ctx.enter_context(
            tc.tile_pool(name="psg", bufs=bufs.get("psg", 8), space="PSUM")
        )

        def one_chunk(b, hc):
            L4 = inp.tile([128, 2, nh * W], mybir.dt.float16, tag="L")
            R4 = inp.tile([128, 2, nh * W], mybir.dt.float16, tag="R")
            if not do_load:  # probe mode: satisfy write-before-read cheaply
                nc.vector.memset(L4[:, :, 0:4], 0.0)
                nc.vector.memset(R4[:, :, 0:4], 0.0)
            if do_load:
                nc.sync.dma_start(
                    L4[:],
                    left[b, :, hc * nh : (hc + 1) * nh, :].rearrange(
                        "(t p) h w -> p t (h w)", p=128
                    ),
                )
                nc.sync.dma_start(
                    R4[:],
                    right[b, :, hc * nh : (hc + 1) * nh, :].rearrange(
                        "(t p) h w -> p t (h w)", p=128
                    ),
                )

            Bt = work.tile([128, nh, win], mybir.dt.float16, tag="B")
            if not do_compute:  # probe mode: satisfy write-before-read
                nc.vector.memset(Bt[:, 0, 0:4], 0.0)
            if do_compute:
                for hl in range(nh):
                    hw0 = hl * W
                    # bank A: j0 at cols [0:168], j1 at [168:376],
                    #         (compaction: j3 at [0:32, 376:448])
                    gA = psg.tile([128, 512], mybir.dt.float32, tag="gA")
                    # bank B: j2 at [0:200]
                    #         (no compaction: j3 folded at [0:32, 128:200])
                    gB = psg.tile([128, 512], mybir.dt.float32, tag="gB")
                    c3 = 376 if compaction else 128
                    for j in range(4):
                        a, n = WINS[j]
                        dst_g = (gA, gA, gB, gA if compaction else gB)[j]
                        c0 = (0, NJ[0], 0, c3)[j]
                        for t in range(2):
                            lhsT = L4[:, t, hw0 + W0S[j] : hw0 + W0S[j] + MS[j]]
                            rhs = R4[:, t, hw0 + a : hw0 + a + n]
                            nc.tensor.matmul(
                                dst_g[0 : MS[j], c0 : c0 + n], lhsT, rhs,
                                start=(t == 0), stop=(t == 1),
                            )
                    if compaction == "g32":
                        for bk, p0, p1, scs, dcs, wd, eng in SLABS32:
                            src = _blocks_ap((gA, gB)[bk][p0:p1, :], scs, wd)
                            dst = _blocks_ap(Bt[p0:p1, hl, :], dcs, wd)
                            if eng == "s":
                                nc.scalar.mul(dst, src, 1.0 / C)
                            else:
                                nc.vector.tensor_scalar_mul(dst, src, 1.0 / C)
                    elif compaction in ("g64", "g64s"):
                        for bk, p0, p1, sc, wd, dc, eng in SLABS64:
                            src = (gA, gB)[bk][p0:p1, sc : sc + wd]
                            dst = Bt[p0:p1, hl, dc : dc + wd]
                            if eng == "s":
                                nc.scalar.mul(dst, src, 1.0 / C)
                            else:
                                nc.vector.tensor_scalar_mul(dst, src, 1.0 / C)
                    else:
                        # bank A (376 cols) on ACT, bank B (200 cols) on DVE
                        nc.scalar.mul(
                            Bt[:, hl, 0 : CUM[2]], gA[:, 0 : CUM[2]], 1.0 / C
                        )
                        nc.vector.tensor_scalar_mul(
                            Bt[:, hl, CUM[2] : WIN], gB[:, 0 : WIN - CUM[2]],
                            1.0 / C,
                        )

            if do_store:
                # issue on the ACT HWDGE ring: its wait (Bt complete) resolves
                # right as ACT's own last eviction retires, so it stalls
                # nothing, and the sync ring stays free for input prefetch
                if compaction in ("g32", "g64s"):
                    # j3's section holds live data only in rows 0:32 -- store
                    # it separately and skip its 96 dead rows (-1.1 MB/core)
                    cj3 = CUM32[3] if compaction == "g32" else CUM64[3]
                    nc.scalar.dma_start(wins_out[b, hc], Bt[:, :, 0:cj3])
                    nc.scalar.dma_start(
                        wins3_out[b, hc], Bt[0:32, :, cj3:win]
                    )
                else:
                    nc.scalar.dma_start(wins_out[b, hc], Bt[:])

        for _rep in range(reps):
            for b in range(B):
                for hc in range(hc_n):
                    one_chunk(b, hc)


def split_dma_waits(nc):
    """Legalize for walrus: instruction descriptors hold ONE sync wait
    (NEURON_ISA_TPB_EVENTS), but Tile attaches up to ~3.  Move the extras to
    standalone waits on the instruction's engine right before it --
    sequencers execute (and enqueue HWDGE descriptors) in program order, so
    the hoisted waits still guard the instruction."""
    n = 0
    for fn in nc.m.functions:
        for bb in fn.blocks:
            insts = bb.instructions
            out = []
            for inst in insts:
                si = getattr(inst, "sync_info", None)
                eng = getattr(inst, "engine", None)
                if (
                    si is not None
                    and si.on_wait
                    and len(si.on_wait) > 1
                    and eng is not None
                    and eng != mybir.EngineType.Unassigned
                ):
                    waits = list(si.on_wait)
                    for w in waits[:-1]:
                        ev = mybir.InstNoOp(name=f"{inst.name}-prewait{n}")
                        ev.engine = eng
                        ev.sync_info = mybir.SyncInfo(on_wait=[w], on_update=[])
                        nc.register_instruction(ev)
                        out.append(ev)
                        n += 1
                    inst.sync_info = mybir.SyncInfo(
                        on_wait=waits[-1:], on_update=list(si.on_update or [])
                    )
                out.append(inst)
            bb.instructions = out
    return n


def build_nc(hs=HS, nh=NH, reps=1, bufs=None, compaction=COMPACTION, **kw):
    if compaction is True:
        compaction = "g64"
    nc = bass.Bass(
        trn_type="TRN2", target_bir_lowering=False, debug=False, num_devices=NCORES
    )
    ins = {
        "left": nc.dram_tensor(
            "left", [B, C, hs, W], mybir.dt.float16, kind="ExternalInput"
        ).ap(),
        "right": nc.dram_tensor(
            "right", [B, C, hs, W], mybir.dt.float16, kind="ExternalInput"
        ).ap(),
    }
    main_win = {"g32": CUM32[3], "g64": WIN64, "g64s": CUM64[3]}.get(
        compaction, WIN
    )
    outs = {
        "wins": nc.dram_tensor(
            "wins",
            [B, hs // nh, 128, nh, main_win],
            mybir.dt.float16,
            kind="ExternalOutput",
        ).ap()
    }
    if compaction in ("g32", "g64s"):
        outs["wins3"] = nc.dram_tensor(
            "wins3",
            [B, hs // nh, 32, nh, 72],
            mybir.dt.float16,
            kind="ExternalOutput",
        ).ap()
    with tile.TileContext(nc) as tc:
        corr_kernel(
            tc, outs, ins, hs=hs, nh=nh, bufs=bufs or BUFS, reps=reps,
            compaction=compaction, **kw
        )
    split_dma_waits(nc)
    return nc


def make_in_maps(left, right):
    in_maps = []
    for i in range(NCORES):
        sl = slice(i * HS, (i + 1) * HS)
        in_maps.append(
            {
                "left": np.ascontiguousarray(left[:, :, sl, :]).astype(np.float16),
                "right": np.ascontiguousarray(right[:, :, sl, :]).astype(np.float16),
            }
        )
    return in_maps


def _deshear_luts(compaction=COMPACTION):
    """Host-side gather LUTs: out[b,d,h,w] = V[b,h, P[w], COL[d,w]] * MASK."""
    w = np.arange(W)
    d = np.arange(D)
    p = w % 128
    if compaction == "g32":
        j = w // 128  # j3 has its own section; p%32 == p there
        col = np.asarray(CUM32)[j][None, :] + (p % 32)[None, :] + d[:, None]
        win = WIN32
    elif compaction in ("g64", "g64s") or compaction is True:
        j = w // 128  # j3 has its own section
        col = np.asarray(CUM64)[j][None, :] + (p % 64)[None, :] + d[:, None]
        win = WIN64
    else:
        j = np.minimum(w // 128, 2)  # j3 lives inside section 2 (cols 128+)
        delta = np.where(j == 0, -MD, 0)
        cum = np.asarray(CUM[:3])
        col = (
            cum[j][None, :] + (w - 128 * j)[None, :] + delta[None, :]
            + d[:, None]
        )
        win = WIN
    r = w[None, :] + d[:, None] - MD
    mask = (r >= 0) & (r < W)
    col = np.clip(col, 0, win - 1)
    return p, col, mask


def deshear(wins_all, hs=HS, nh=NH, compaction=COMPACTION, wins3_all=None):
    """wins_all: [ncores, B, hs//nh, 128, nh, win] fp16 -> [B, D, H, W] f32.

    For g32, wins_all is the [.., 336]-wide main store and wins3_all the
    [.., 32, nh, 72] j3 store; they are re-joined into a [.., 128, 408] view.
    """
    ncores, nb = wins_all.shape[0], wins_all.shape[1]
    if compaction in ("g32", "g64s"):
        assert wins3_all is not None
        hc_n = wins_all.shape[2]
        tot = WIN32 if compaction == "g32" else WIN64
        cj3 = tot - 72
        full = np.empty((ncores, nb, hc_n, 128, nh, tot), wins_all.dtype)
        full[..., 0:cj3] = wins_all
        full[:, :, :, 0:32, :, cj3:] = wins3_all
        wins_all = full
    win = wins_all.shape[-1]
    p, col, mask = _deshear_luts(compaction)
    v = wins_all.transpose(0, 1, 2, 4, 3, 5).reshape(ncores, nb, hs, 128, win)
    pb = np.broadcast_to(p[None, :], (D, W))
    res = v[:, :, :, pb, col]  # [ncores, nb, hs, D, W]
    # np.where (not multiply): masked-off slots may hold uninitialized SBUF
    # garbage, which can be NaN
    res = np.where(mask[None, None, None], res.astype(np.float32), 0.0)
    return res.transpose(1, 3, 0, 2, 4).reshape(nb, D, ncores * hs, W)


def kernel(left, right):
    """Full-input entry point: [4,256,128,416] fp32 x2 -> [4,81,128,416] fp32."""
    from concourse.bass_utils import run_bass_kernel_spmd

    left = np.asarray(left, dtype=np.float32)
    right = np.asarray(right, dtype=np.float32)
    nc = build_nc()
    in_maps = make_in_maps(left, right)
    res = run_bass_kernel_spmd(nc, in_maps, list(range(NCORES)))
    wins_all = np.stack([res.results[i]["wins"] for i in range(NCORES)])
    wins3_all = None
    if COMPACTION in ("g32", "g64s"):
        wins3_all = np.stack([res.results[i]["wins3"] for i in range(NCORES)])
    return deshear(wins_all, wins3_all=wins3_all)


if __name__ == "__main__":
    rng = np.random.default_rng(0)
    lf = rng.standard_normal((B, C, H, W), dtype=np.float32)
    rt = rng.standard_normal((B, C, H, W), dtype=np.float32)
    o = kernel(left=lf, right=rt)
    print(o.shape, o.dtype)
